# revision 6
# baseline (speedup 1.0000x reference)
"""Multi-head attention (B=2, S=2048, D=1024, H=16, causal mask) on 8 TRN2
NeuronCores, head-parallel: each core computes 2 heads' q/k/v + attention and
a partial output projection; host sums the 8 partials and adds bo.

Layouts (per core):
  xT      (1024, 4096)  feature-major tokens (b-major), replicated
  wqkvT   (1024, 384)   [wq(/8) | wk | wv] columns for this core's 2 heads
  bqkv    (128, 3)      per-dim biases (bq/8, bk, bv)
  woT     (128, 1024)   wo rows for this core's head dims
  out     (4096, 1024)  partial x @ ... contribution (host sums over cores)

All matmuls run as float32r (TF32-like, full PE rate at moving dim >= 256).
Softmax skips max-subtraction (scores bounded ~10 for this problem's scale)
and defers normalization: P = exp(S^T) unnormalized, denominators come from a
ones-column appended to v, and the 1/denom scale is applied after the
(per-head, row-tiled) output projection.
"""

import numpy as np

import concourse.bass as bass
import concourse.tile as tile
from concourse import bacc, mybir
from concourse.bass_utils import run_bass_kernel_spmd

B, S, D, H = 2, 2048, 1024, 16
DH = D // H  # 64
NCORES = 8
HPC = H // NCORES  # 2 heads per core
T = B * S  # 4096
QCH = 512  # q-chunk (moving dim)
KCH = 128  # k-chunk (stationary dim)
NQC = S // QCH  # 4 per batch
NKC = S // KCH  # 16 per batch
NTC = T // QCH  # 8 token chunks overall
ND = D // 128  # 8 feature chunks

f32 = mybir.dt.float32
f32r = mybir.dt.float32r
AF = mybir.ActivationFunctionType
ALU = mybir.AluOpType

# score/PV software pipeline depth (PE issues scores k blocks ahead of PV)
PIPE = 2


def _classify_blocks(mask):
    """mask: (S, S) bool [q, k]. Returns dict (qc, kc) -> ('none'|'all'|'causal'|'mixed', packed_idx)."""
    blocks = {}
    qg, kg = np.meshgrid(np.arange(S), np.arange(S), indexing="ij")
    causal = qg >= kg
    n_mixed = 0
    for qc in range(NQC):
        for kc in range(NKC):
            reg = mask[qc * QCH : (qc + 1) * QCH, kc * KCH : (kc + 1) * KCH]
            if not reg.any():
                blocks[(qc, kc)] = ("none", -1)
            elif reg.all():
                blocks[(qc, kc)] = ("all", -1)
            elif np.array_equal(
                reg, causal[qc * QCH : (qc + 1) * QCH, kc * KCH : (kc + 1) * KCH]
            ):
                blocks[(qc, kc)] = ("causal", -1)
            else:
                blocks[(qc, kc)] = ("mixed", n_mixed)
                n_mixed += 1
    return blocks, n_mixed


def _build(mask):
    blocks, n_mixed = _classify_blocks(mask)

    nc = bacc.Bacc("TRN2", target_bir_lowering=False, debug=False, num_devices=NCORES)
    xt_d = nc.dram_tensor("xt", (D, T), f32r, kind="ExternalInput").ap()
    w_d = nc.dram_tensor("wqkv", (D, 3 * 128), f32r, kind="ExternalInput").ap()
    b_d = nc.dram_tensor("bqkv", (128, 3), f32, kind="ExternalInput").ap()
    wo_d = nc.dram_tensor("wot", (128, D), f32r, kind="ExternalInput").ap()
    id_d = nc.dram_tensor("ident", (128, 64), f32r, kind="ExternalInput").ap()
    out_d = nc.dram_tensor("out", (T, D), f32, kind="ExternalOutput").ap()
    if n_mixed:
        mb_d = nc.dram_tensor(
            "mblk", (n_mixed * 128, QCH), f32r, kind="ExternalInput"
        ).ap()

    with tile.TileContext(nc) as tc:
        with (
            tc.tile_pool(name="const", bufs=1) as cpool,
            tc.tile_pool(name="act", bufs=1) as apool,
            tc.tile_pool(name="work", bufs=1) as wpool,
            tc.tile_pool(name="psum", bufs=1, space="PSUM") as ppool,
        ):
            # ---- constants ----
            w8 = cpool.tile([128, ND * 384], f32r)  # chunk dc at [:, dc*384:+384]
            nc.sync.dma_start(
                w8[:].rearrange("p (c f) -> p c f", c=ND),
                w_d.rearrange("(c p) f -> p c f", p=128),
            )
            wot = cpool.tile([128, D], f32r)
            nc.sync.dma_start(wot[:], wo_d)
            bqkv = cpool.tile([128, 3], f32)
            nc.sync.dma_start(bqkv[:], b_d)
            ident = cpool.tile([128, 64], f32r)
            nc.sync.dma_start(ident[:], id_d)

            # ---- per-batch persistent activations ----
            # qT/kT: [128 dims(2 heads), S tokens]; v_aug: per head [128 ktok, 16*65]
            qT = [apool.tile([128, S], f32r, tag=f"qT{b}", name=f"qT{b}") for b in range(B)]
            kT = [apool.tile([128, S], f32r, tag=f"kT{b}", name=f"kT{b}") for b in range(B)]
            vaug = [
                [apool.tile([128, NKC * 65], f32r, tag=f"va{b}{h}", name=f"va{b}{h}") for h in range(HPC)]
                for b in range(B)
            ]
            ones16 = cpool.tile([128, NKC], f32)
            nc.vector.memset(ones16[:], 1.0)
            for b in range(B):
                for h in range(HPC):
                    nc.vector.tensor_copy(vaug[b][h][:, 64 :: 65], ones16[:])

            # ---- phase A: qkv projections + v transpose ----
            for t in range(NTC):
                b, tq = t // NQC, t % NQC  # batch, within-batch 512-chunk
                xts = []
                for dc in range(ND):
                    xt = wpool.tile([128, QCH], f32r, tag="x", bufs=16)
                    nc.sync.dma_start(
                        xt[:], xt_d[dc * 128 : (dc + 1) * 128, t * QCH : (t + 1) * QCH]
                    )
                    xts.append(xt)
                for p in range(3):  # q, k, v
                    ps = ppool.tile([128, QCH], f32, tag="st", bufs=4)
                    for dc in range(ND):
                        nc.tensor.matmul(
                            ps[:],
                            w8[:, dc * 384 + p * 128 : dc * 384 + (p + 1) * 128],
                            xts[dc][:],
                            start=(dc == 0),
                            stop=(dc == ND - 1),
                        )
                    if p == 0:
                        dst = qT[b][:, tq * QCH : (tq + 1) * QCH]
                    elif p == 1:
                        dst = kT[b][:, tq * QCH : (tq + 1) * QCH]
                    else:
                        dst = wpool.tile([128, QCH], f32r, tag="vst", bufs=2, name=f"vst{t}")[:]
                    nc.scalar.activation(
                        dst, ps[:], AF.Identity, bias=bqkv[:, p : p + 1], scale=1.0
                    )
                    if p == 2:
                        # transpose v to natural [ktok, dim] per head
                        for j in range(QCH // 128):
                            kc = tq * 4 + j
                            for h in range(HPC):
                                tp = ppool.tile([128, 512], f32r, tag="acc", bufs=4, name=f"vt{t}_{j}_{h}")
                                nc.tensor.transpose(
                                    tp[:, 0:64],
                                    dst[h * 64 : (h + 1) * 64, j * 128 : (j + 1) * 128],
                                    ident[h * 64 : (h + 1) * 64, :],
                                )
                                nc.vector.tensor_copy(
                                    vaug[b][h][:, kc * 65 : kc * 65 + 64], tp[:, 0:64]
                                )

            # ---- phases B/C per (batch, q-chunk) ----
            for b in range(B):
                for qc in range(NQC):
                    kcs = [
                        kc for kc in range(NKC) if blocks[(qc, kc)][0] != "none"
                    ]
                    acc = [
                        ppool.tile([128, QCH], f32, tag="acc", bufs=4, name=f"acc{b}_{qc}_{h}")
                        for h in range(HPC)
                    ]
                    pts = {}

                    def emit_scores(i):
                        kc = kcs[i]
                        kind, midx = blocks[(qc, kc)]
                        for h in range(HPC):
                            st = ppool.tile([128, QCH], f32, tag="st", bufs=4)
                            nc.tensor.matmul(
                                st[:],
                                kT[b][h * 64 : (h + 1) * 64, kc * KCH : (kc + 1) * KCH],
                                qT[b][h * 64 : (h + 1) * 64, qc * QCH : (qc + 1) * QCH],
                                start=True,
                                stop=True,
                                tile_position=(h * 64, 0),
                            )
                            pt = wpool.tile([128, QCH], f32r, tag="pt", bufs=8)
                            nc.scalar.activation(pt[:], st[:], AF.Exp)
                            if kind == "causal":
                                nc.gpsimd.affine_select(
                                    out=pt[:],
                                    in_=pt[:],
                                    compare_op=ALU.is_ge,
                                    fill=0.0,
                                    base=qc * QCH - kc * KCH,
                                    pattern=[[1, QCH]],
                                    channel_multiplier=-1,
                                )
                            elif kind == "mixed":
                                mt = wpool.tile([128, QCH], f32r, tag="mt", bufs=4)
                                nc.sync.dma_start(
                                    mt[:], mb_d[midx * 128 : (midx + 1) * 128, :]
                                )
                                nc.vector.tensor_mul(pt[:], pt[:], mt[:])
                            pts[(i, h)] = pt

                    def emit_pv(i):
                        kc = kcs[i]
                        for h in range(HPC):
                            nc.tensor.matmul(
                                acc[h][0:65, :],
                                vaug[b][h][:, kc * 65 : (kc + 1) * 65],
                                pts.pop((i, h))[:],
                                start=(i == 0),
                                stop=(i == len(kcs) - 1),
                            )

                    for i in range(len(kcs)):
                        emit_scores(i)
                        if i >= PIPE:
                            emit_pv(i - PIPE)
                    for i in range(max(0, len(kcs) - PIPE), len(kcs)):
                        emit_pv(i)

                    # unnormalized attn^T + denominators
                    attnT = wpool.tile([128, QCH], f32r, tag="attnT", bufs=2)
                    recips = []
                    for h in range(HPC):
                        nc.vector.tensor_copy(
                            attnT[h * 64 : (h + 1) * 64, :], acc[h][0:64, :]
                        )
                        denom = wpool.tile(
                            [1, QCH], f32, tag=f"denom{h}", bufs=2, name=f"denom{h}"
                        )
                        nc.vector.tensor_copy(denom[:], acc[h][64:65, :])
                        recip = wpool.tile(
                            [1, QCH], f32, tag=f"recip{h}", bufs=2, name=f"recip{h}"
                        )
                        nc.vector.reciprocal(recip[:], denom[:])
                        recips.append(recip)
                    # transpose scales to token-major: sT[p, tk*2+h] = recip[h, tk*128+p]
                    sT = wpool.tile([128, 2 * (QCH // 128)], f32, tag="sT", bufs=2)
                    for h in range(HPC):
                        for tk in range(QCH // 128):
                            nc.sync.dma_start(
                                sT[:, (tk * 2 + h) : (tk * 2 + h + 1)],
                                recips[h][
                                    0:1, tk * 128 : (tk + 1) * 128
                                ].rearrange("o (p x) -> o p x", x=1),
                            )

                    # ---- phase C: per-head out-proj (row-tiled), scale+combine ----
                    for tk in range(QCH // 128):
                        for oc in range(D // QCH):
                            ops = [
                                ppool.tile([128, QCH], f32, tag="st", bufs=4, name=f"op{b}_{qc}_{tk}_{oc}_{h}")
                                for h in range(HPC)
                            ]
                            for h in range(HPC):
                                nc.tensor.matmul(
                                    ops[h][:],
                                    attnT[
                                        h * 64 : (h + 1) * 64,
                                        tk * 128 : (tk + 1) * 128,
                                    ],
                                    wot[h * 64 : (h + 1) * 64, oc * QCH : (oc + 1) * QCH],
                                    start=True,
                                    stop=True,
                                    tile_position=(h * 64, 0),
                                )
                            tmp = wpool.tile([128, QCH], f32, tag="tmp", bufs=3)
                            nc.scalar.activation(
                                tmp[:],
                                ops[0][:],
                                AF.Copy,
                                scale=sT[:, tk * 2 : tk * 2 + 1],
                            )
                            osb = wpool.tile([128, QCH], f32, tag="osb", bufs=3)
                            nc.vector.scalar_tensor_tensor(
                                osb[:],
                                ops[1][:],
                                sT[:, tk * 2 + 1 : tk * 2 + 2],
                                tmp[:],
                                ALU.mult,
                                ALU.add,
                            )
                            row0 = b * S + qc * QCH + tk * 128
                            nc.sync.dma_start(
                                out_d[row0 : row0 + 128, oc * QCH : (oc + 1) * QCH],
                                osb[:],
                            )

    nc.compile()
    return nc, blocks, n_mixed


_CACHE = {}


def _get_program(mask):
    key = mask.tobytes()
    if key not in _CACHE:
        _CACHE[key] = _build(mask)
    return _CACHE[key]


def kernel(x, mask, wq, bq, wk, bk, wv, bv, wo, bo):
    x = np.asarray(x, dtype=np.float32)
    mask2 = np.asarray(mask).reshape(S, S)
    nc, blocks, n_mixed = _get_program(mask2)

    xT = np.ascontiguousarray(x.reshape(T, D).T)
    ident = np.ascontiguousarray(np.tile(np.eye(64, dtype=np.float32), (2, 1)))

    if n_mixed:
        mb = np.zeros((n_mixed * 128, QCH), dtype=np.float32)
        for (qc, kc), (kind, midx) in blocks.items():
            if kind == "mixed":
                reg = mask2[qc * QCH : (qc + 1) * QCH, kc * KCH : (kc + 1) * KCH]
                mb[midx * 128 : (midx + 1) * 128, :] = reg.T.astype(np.float32)

    in_maps = []
    for c in range(NCORES):
        hsl = slice(c * HPC * DH, (c + 1) * HPC * DH)
        wqkv = np.concatenate(
            [
                np.asarray(wq)[hsl, :].T / np.sqrt(DH),
                np.asarray(wk)[hsl, :].T,
                np.asarray(wv)[hsl, :].T,
            ],
            axis=1,
        ).astype(np.float32)
        bqkv = np.stack(
            [
                np.asarray(bq)[hsl] / np.sqrt(DH),
                np.asarray(bk)[hsl],
                np.asarray(bv)[hsl],
            ],
            axis=1,
        ).astype(np.float32)
        m = {
            "xt": xT,
            "wqkv": np.ascontiguousarray(wqkv),
            "bqkv": np.ascontiguousarray(bqkv),
            "wot": np.ascontiguousarray(np.asarray(wo)[:, hsl].T.astype(np.float32)),
            "ident": ident,
        }
        if n_mixed:
            m["mblk"] = mb
        in_maps.append(m)

    res = run_bass_kernel_spmd(nc, in_maps, core_ids=list(range(NCORES)))
    out = res.results[0]["out"].astype(np.float64)
    for c in range(1, NCORES):
        out += res.results[c]["out"]
    out = (out + np.asarray(bo)).astype(np.float32)
    return out.reshape(B, S, D)
